# revision 6
# baseline (speedup 1.0000x reference)
"""Additive (Bahdanau) alignment kernel for Trainium2, SPMD across 8 NeuronCores.

Model (per batch row b):
    dec_p = decoder_output @ W_dec.T + b_dec                  # (A,)
    enc_p = encoder_output[b] @ W_enc.T + b_enc               # (S, A)
    h     = tanh(dec_p + enc_p)                               # (S, A)
    scores= h @ V.T + b_v                                     # (S,)
    attn  = softmax(scores)                                   # (S,)
    ctx   = attn @ encoder_output[b]                          # (H,)
    out   = concat(ctx, decoder_output[b])                    # (2H,)

Strategy: data-parallel over batch (8 rows per core).  Single pass over the
encoder (read from HBM exactly once, cast f32->bf16 during the DMA).  Scores
are bounded (|scores| <= sum|V| + |b_v| ~ 23) so softmax needs no max
subtraction: accumulate ctx_unnorm = sum_s exp(score_s) * enc_s and
l = sum_s exp(score_s) in one pass, normalize at the end.

Per seq tile of 512 (x4 per batch):
  - DMA-cast enc tile -> SBUF bf16 [128, 4(sub), 512(h)]
  - PE-transpose -> encT [128(h), 4(hc), 512(s)]  (evacuated via DVE)
  - 16 matmuls: enc_pT[a_chunk, s] += W_encT[hc,ac] @ encT[hc]   (PSUM f32)
  - ScalarE tanh with per-partition bias = dec_p[a] + b_dec[a] + b_enc[a]
  - 4 matmuls: scores[1, s] += V[ac] @ hT[ac]
  - ScalarE exp (bias=b_v) with accum_out -> l partial
  - PE-transpose e row -> e column; 4 matmuls: ctx[1, h] += e_col.T @ enc
"""

import numpy as np
from contextlib import ExitStack

import concourse.bass as bass
import concourse.mybir as mybir
import concourse.tile as tile
from concourse.vector_clock import ScopedClock
from concourse.masks import make_identity
from concourse.bass import ts
from concourse.bass_utils import run_bass_kernel_spmd

F32 = mybir.dt.float32
BF16 = mybir.dt.bfloat16
AF = mybir.ActivationFunctionType

N_CORES = 8
B, S, H, A = 64, 2048, 512, 512
B_SH = B // N_CORES


class _SplitDrainTileContext(tile.TileContext):
    """This walrus build rejects instructions carrying more than a couple of
    semaphore waits ("Too many sync wait commands").  The stock TileContext
    tail puts every outstanding proc's wait on one Drain; split them across
    single-wait NOPs instead."""

    def _drain_and_barrier(self, tick_clock, wait_clock):
        nc = self.nc
        drain_inst = nc.sync.drain()
        wait_clock.add_sem_waits(
            drain_inst.ins, ScopedClock({None: tick_clock.global_clock})
        )
        si = drain_inst.ins.sync_info
        waits = list(si.on_wait)
        if len(waits) > 1:
            drain_inst.ins.sync_info = mybir.SyncInfo(
                on_wait=[waits[0]], on_update=list(si.on_update)
            )
            for w in waits[1:]:
                nop = nc.sync.nop(nofuse=True)
                nop.ins.sync_info = mybir.SyncInfo(on_wait=[w], on_update=[])

        nc.all_engine_barrier()
        assert self.sems is not None
        popped = nc._tile_sem_poison_stack.pop()
        assert popped is self._sem_poison
        nc.clear_and_free_semaphores(list(self.sems.allocated().values()))
        nc.all_engine_barrier()


def _split_excess_waits(nc, max_waits=1):
    """walrus (this build) rejects instructions with more than a couple of
    semaphore waits.  Move excess waits onto single-wait NOPs inserted just
    before the offending instruction on the same engine."""
    for fn in nc.m.functions:
        for bb in fn.blocks:
            new_insts = []
            for inst in bb.instructions:
                si = inst.sync_info
                waits = list(si.on_wait) if si is not None else []
                if len(waits) > max_waits:
                    head, keep = waits[:-max_waits], waits[-max_waits:]
                    for i, w in enumerate(head):
                        nop = mybir.InstNoOp(
                            name=f"{inst.name}-sw{i}",
                            engine=inst.engine,
                            bass_nofuse=True,
                            sync_info=mybir.SyncInfo(on_wait=[w], on_update=[]),
                        )
                        nc.register_instruction(nop, overwrite=True)
                        new_insts.append(nop)
                    inst.sync_info = mybir.SyncInfo(
                        on_wait=keep, on_update=list(si.on_update)
                    )
                new_insts.append(inst)
            bb.instructions[:] = new_insts


def build_nc(b_sh=B_SH, s=S, h=H, a=A, st=512):
    """Build the per-core Bass graph (SPMD: same graph on all cores)."""
    assert h % 128 == 0 and a % 128 == 0 and st % 128 == 0 and s % st == 0
    HC = h // 128  # h chunks
    AC = a // 128  # a chunks
    SUB = st // 128  # 128-row subtiles per seq tile
    NT = s // st  # seq tiles per batch row

    nc = bass.Bass("TRN2", target_bir_lowering=False, debug=False)
    dec = nc.declare_dram_parameter("decoder_output", (b_sh, h), F32, isOutput=False)
    enc = nc.declare_dram_parameter("encoder_output", (b_sh, s, h), F32, isOutput=False)
    Wd = nc.declare_dram_parameter("W_dec", (a, h), F32, isOutput=False)
    bd = nc.declare_dram_parameter("b_dec", (a,), F32, isOutput=False)
    We = nc.declare_dram_parameter("W_enc", (a, h), F32, isOutput=False)
    be = nc.declare_dram_parameter("b_enc", (a,), F32, isOutput=False)
    V = nc.declare_dram_parameter("V", (1, a), F32, isOutput=False)
    bv = nc.declare_dram_parameter("b_v", (1,), F32, isOutput=False)
    out = nc.declare_dram_parameter("out", (b_sh, 2 * h), F32, isOutput=True)

    with ExitStack() as ctx:
        tc = ctx.enter_context(_SplitDrainTileContext(nc))

        consts = ctx.enter_context(tc.tile_pool(name="consts", bufs=1))

        ident = consts.tile([128, 128], BF16)
        make_identity(nc, ident)

        # ---- weight / decoder prep (PSUM pools scoped so banks are freed) ----
        WeT = consts.tile([128, HC * AC, 128], BF16)  # [h_in, hc*AC+ac, a_in]
        WdT = consts.tile([128, HC * AC, 128], BF16)
        dterm = consts.tile([128, AC, b_sh], F32)  # dec_p + b_dec + b_enc, [a_in, ac, b]
        Vc = consts.tile([128, AC], BF16)  # V as columns  [a_in, ac]
        bvt = consts.tile([1, 1], F32)

        with (
            tc.tile_pool(name="wps", bufs=2, space="PSUM") as wps_pool,
            tc.tile_pool(name="wtmp", bufs=2) as wtmp_pool,
        ):
            for Wsrc, Wdst in ((We, WeT), (Wd, WdT)):
                for asub in range(AC):
                    wt = wtmp_pool.tile([128, h], BF16, tag="wt")
                    nc.gpsimd.dma_start(out=wt, in_=Wsrc[ts(asub, 128), :])
                    wp = wps_pool.tile([128, HC, 128], BF16, tag="wp")
                    for hc in range(HC):
                        nc.tensor.transpose(wp[:, hc, :], wt[:, ts(hc, 128)], ident)
                    dstv = Wdst.rearrange("p (hc ac) f -> p hc ac f", ac=AC)[
                        :, :, asub, :
                    ]
                    nc.vector.tensor_copy(dstv, wp)

            # decoder projection dec_p[b, a] plus biases -> dterm
            dec_bf = wtmp_pool.tile([b_sh, h], BF16, tag="decbf")
            nc.gpsimd.dma_start(out=dec_bf, in_=dec[:, :])
            decT_ps = wps_pool.tile([128, HC, b_sh], BF16, tag="wp")
            for hc in range(HC):
                nc.tensor.transpose(
                    decT_ps[:, hc, :], dec_bf[:, ts(hc, 128)], ident[:b_sh, :b_sh]
                )
            decT = wtmp_pool.tile([128, HC, b_sh], BF16, tag="decT")
            nc.vector.tensor_copy(decT, decT_ps)

            bias_d = wtmp_pool.tile([128, AC], F32, tag="biasd")
            bias_e = wtmp_pool.tile([128, AC], F32, tag="biase")
            nc.sync.dma_start(out=bias_d, in_=bd.rearrange("(c p) -> p c", p=128))
            nc.sync.dma_start(out=bias_e, in_=be.rearrange("(c p) -> p c", p=128))
            bias2 = wtmp_pool.tile([128, AC], F32, tag="bias2")
            nc.vector.tensor_add(bias2, bias_d, bias_e)

            for ac in range(AC):
                dt_ps = wps_pool.tile([128, b_sh], F32, tag="dtps")
                for hc in range(HC):
                    nc.tensor.matmul(
                        dt_ps,
                        WdT[:, hc * AC + ac, :],
                        decT[:, hc, :],
                        start=(hc == 0),
                        stop=(hc == HC - 1),
                    )
                nc.vector.tensor_scalar_add(dterm[:, ac, :], dt_ps, bias2[:, ac : ac + 1])

            nc.gpsimd.dma_start(out=Vc, in_=V[0, :].rearrange("(c p) -> p c", p=128))
            nc.sync.dma_start(out=bvt, in_=bv[None, :])

        # ---- main loop pools ----
        enc_pool = ctx.enter_context(tc.tile_pool(name="enc", bufs=3))
        encT_sb_pool = ctx.enter_context(tc.tile_pool(name="encTsb", bufs=2))
        hT_pool = ctx.enter_context(tc.tile_pool(name="hT", bufs=3))
        erow_pool = ctx.enter_context(tc.tile_pool(name="erow", bufs=2))
        ecol_pool = ctx.enter_context(tc.tile_pool(name="ecol", bufs=2))
        small_pool = ctx.enter_context(tc.tile_pool(name="small", bufs=2))

        # PSUM budget (8 banks): encT 2 + encp 2 + scores 1 + ecol 1 + ctx 2
        encT_ps_pool = ctx.enter_context(tc.tile_pool(name="encTps", bufs=2, space="PSUM"))
        encp_ps_pool = ctx.enter_context(tc.tile_pool(name="encpps", bufs=2, space="PSUM"))
        scores_ps_pool = ctx.enter_context(tc.tile_pool(name="scoresps", bufs=1, space="PSUM"))
        ecol_ps_pool = ctx.enter_context(tc.tile_pool(name="ecolps", bufs=1, space="PSUM"))
        ctx_ps_pool = ctx.enter_context(tc.tile_pool(name="ctxps", bufs=2, space="PSUM"))

        for b in range(b_sh):
            ctx_ps = ctx_ps_pool.tile([1, h], F32, tag="ctx")
            lparts = small_pool.tile([1, NT], F32, tag="lparts")
            for t in range(NT):
                et = enc_pool.tile([128, SUB, h], BF16, tag="et")
                nc.gpsimd.dma_start(
                    out=et,
                    in_=enc[b, ts(t, st), :].rearrange("(sub p) h -> p sub h", p=128),
                )
                encT_sb = encT_sb_pool.tile([128, HC, st], BF16, tag="encT")
                for hc in range(HC):
                    etp = encT_ps_pool.tile([128, st], BF16, tag="etp")
                    for sub in range(SUB):
                        nc.tensor.transpose(
                            etp[:, ts(sub, 128)], et[:, sub, ts(hc, 128)], ident
                        )
                    nc.vector.tensor_copy(encT_sb[:, hc, :], etp)

                scores_ps = scores_ps_pool.tile([1, st], F32, tag="scores")
                for ac in range(AC):
                    pp = encp_ps_pool.tile([128, st], F32, tag="pp")
                    for hc in range(HC):
                        nc.tensor.matmul(
                            pp,
                            WeT[:, hc * AC + ac, :],
                            encT_sb[:, hc, :],
                            start=(hc == 0),
                            stop=(hc == HC - 1),
                        )
                    hT = hT_pool.tile([128, st], BF16, tag="hT")
                    nc.scalar.activation(
                        out=hT,
                        in_=pp,
                        func=AF.Tanh,
                        bias=dterm[:, ac, b : b + 1],
                        scale=1.0,
                    )
                    nc.tensor.matmul(
                        scores_ps,
                        Vc[:, ac : ac + 1],
                        hT,
                        start=(ac == 0),
                        stop=(ac == AC - 1),
                    )

                erow = erow_pool.tile([1, st], BF16, tag="erow")
                nc.scalar.activation(
                    out=erow,
                    in_=scores_ps,
                    func=AF.Exp,
                    bias=bvt,
                    scale=1.0,
                    accum_out=lparts[:, t : t + 1],
                )
                # bf16 PSUM writes must land on 4-byte boundaries: use every
                # other column of a [128, 2*SUB] tile.
                ecol_ps = ecol_ps_pool.tile([128, 2 * SUB], BF16, tag="ecolps")
                for sub in range(SUB):
                    nc.tensor.transpose(
                        ecol_ps[:, 2 * sub : 2 * sub + 1],
                        erow[:, ts(sub, 128)],
                        ident[:1, :1],
                    )
                ecol = ecol_pool.tile([128, SUB], BF16, tag="ecol")
                nc.vector.tensor_copy(ecol, ecol_ps[:, 0 : 2 * SUB : 2])
                for sub in range(SUB):
                    nc.tensor.matmul(
                        ctx_ps,
                        ecol[:, sub : sub + 1],
                        et[:, sub, :],
                        start=(t == 0 and sub == 0),
                        stop=(t == NT - 1 and sub == SUB - 1),
                    )

            lsum = small_pool.tile([1, 1], F32, tag="lsum")
            nc.vector.reduce_sum(lsum, lparts, mybir.AxisListType.X)
            linv = small_pool.tile([1, 1], F32, tag="linv")
            nc.vector.reciprocal(linv, lsum)
            orow = small_pool.tile([1, h], F32, tag="orow")
            nc.vector.tensor_scalar_mul(orow, ctx_ps, linv)
            nc.sync.dma_start(out=out[b : b + 1, 0:h], in_=orow)

        # decoder passthrough: out[:, h:] = decoder_output
        nc.sync.dma_start(out=out[:, h : 2 * h], in_=dec[:, :])

    _split_excess_waits(nc)
    return nc


_CACHED = {}


def _get_nc():
    if "nc" not in _CACHED:
        _CACHED["nc"] = build_nc()
    return _CACHED["nc"]


def kernel(**inputs) -> np.ndarray:
    ins = {
        k: np.ascontiguousarray(np.asarray(v, dtype=np.float32)) for k, v in inputs.items()
    }
    nc = _get_nc()
    in_maps = []
    for c in range(N_CORES):
        sl = slice(c * B_SH, (c + 1) * B_SH)
        in_maps.append(
            {
                "decoder_output": ins["decoder_output"][sl],
                "encoder_output": ins["encoder_output"][sl],
                "W_dec": ins["W_dec"],
                "b_dec": ins["b_dec"],
                "W_enc": ins["W_enc"],
                "b_enc": ins["b_enc"],
                "V": ins["V"],
                "b_v": ins["b_v"],
            }
        )
    res = run_bass_kernel_spmd(nc, in_maps, core_ids=list(range(N_CORES)))
    return np.concatenate([res.results[c]["out"] for c in range(N_CORES)], axis=0)
